# revision 1
# baseline (speedup 1.0000x reference)
"""Trainium2 Bass kernel for nn_BagInput (segment_reduce), v2.

Pipeline per core (data-parallel over contiguous segment ranges):
  h    = x_aug @ W_aug            (PE; x host-pretransposed to [81, items] f16)
  r    = relu(h)                  (ACT activation Relu + DVE tensor_scalar_max,
                                   split; only these engines can read PSUM)
  seg  = 0.99*segsum(r) + 0.01*segsum(h)
       = A-matmul (0/0.99 blob) + W @ xsumT (host per-seg sums)   (PE)
  out  = LayerNorm(seg)           (evict f16 -> DMA-transpose -> bn_stats DVE
                                   -> sqrt ACT -> recip DVE -> normalize Pool)

LayerNorm is scale-invariant per row, so segment SUMS replace MEANS (the
1/len cancels; eps perturbation <= 2e-4 relative). lrelu decomposes as
lrelu(h) = 0.01*h + 0.99*relu(h); segsum(h) is linear in the per-segment
input sums, computed on host and folded in via one matmul per block.
"""
import sys
import os

sys.path.insert(0, "/opt/trn_rl_repo")

import numpy as np
import orjson

import concourse.bass as bass
import concourse.tile as tile
from concourse import mybir
from concourse.bass_utils import run_bass_kernel_spmd

FEAT = 64
NMASK = 16
FDIM = FEAT + NMASK + 1  # 81: feats + mask + ones column (bias)
BAG = 128
LEAK = 0.01
LN_EPS = 1e-5
NCORES = 8
TILE = 128            # items per tile (partition dim)
GROUP = 12            # tiles per group (one relu op / one PSUM h tile)
CHUNKG = 4            # groups per input-DMA chunk
SEGBLK = 512          # segments per psum block
RELU_SCALE = float(np.float16(1.0 - LEAK))  # folded into A blob (f16-exact)
F16 = mybir.dt.float16
F32 = mybir.dt.float32

DEFER_MM2 = 3         # groups between mm1/relu emission and mm2 emission
DEFER_F1 = 2          # extra groups before evict+transpose of a closed block
DEFER_F2 = 8          # extra groups before stats/aggr of a block
DEFER_F3 = 12         # extra groups before sqrt/recip/normalize/out
OUT_BATCH = 4         # blocks per output DMA

# relu engine split (ACT : DVE), and x-chunk DMA queue pattern
RELU_ACT_SHARE = 0.555
XQ_PATTERN = ("g", "g", "g", "s", "g", "s", "g", "s")  # s=sync g=gpsimd


# ---------------------------------------------------------------------------
# BIR post-pass: this container's neuronxcc walrus accepts only ONE sync-wait
# per instruction; Tile attaches several.  Waiting on monotonic semaphores
# one-at-a-time in program order on the same engine is equivalent.
def _split_multi_waits(bir_bytes: bytes) -> bytes:
    mod = orjson.loads(bir_bytes)
    n = 0
    for fn in mod["functions"]:
        for bb in fn["blocks"]:
            out = []
            for ins in bb["instructions"]:
                si = ins.get("sync_info")
                waits = si.get("on_wait") if si else None
                if waits and len(waits) > 1:
                    for w in waits[:-1]:
                        n += 1
                        nop = {
                            "engine": ins["engine"],
                            "ins": [],
                            "name": f"WSPLIT-{n}",
                            "opcode": "NoOp",
                            "outs": [],
                            "sync_info": {"on_update": [], "on_wait": [w]},
                        }
                        if "debug" in ins:
                            nop["debug"] = ins["debug"]
                        out.append(nop)
                    si["on_wait"] = [waits[-1]]
                out.append(ins)
            bb["instructions"] = out
    return orjson.dumps(mod)


def _patch_bass(nc):
    orig = nc.to_json_bytes
    nc.to_json_bytes = lambda: _split_multi_waits(orig())
    return nc


# ---------------------------------------------------------------------------
# Host-side structure: per-core segment pieces for the segment-sum matmuls.
def _build_structure(x_len_core: np.ndarray, i_pad: int):
    """Pieces: [tile, block, psum_off, width, a_off, start, final] per
    128-item tile; windows cover every column of every 512-seg block.
    A entries are RELU_SCALE (0/0.99 indicator; LN makes 1/len unnecessary)."""
    x_len_core = x_len_core.astype(np.int64)
    n_items = int(x_len_core.sum())
    n_seg = len(x_len_core)
    nblk = (n_seg + SEGBLK - 1) // SEGBLK
    seg_of_item = np.repeat(np.arange(n_seg), x_len_core)
    t_pad = i_pad // TILE

    raw = []  # [tile, block, lo, hi) in block-local cols
    for t in range(t_pad):
        lo_i, hi_i = t * TILE, min((t + 1) * TILE, n_items)
        if lo_i >= n_items:
            break
        s0, s1 = int(seg_of_item[lo_i]), int(seg_of_item[hi_i - 1])
        for b in range(s0 // SEGBLK, s1 // SEGBLK + 1):
            sa, sb = max(s0, b * SEGBLK), min(s1, b * SEGBLK + SEGBLK - 1)
            raw.append([t, b, sa - b * SEGBLK, sb - b * SEGBLK + 1])

    # coverage: first piece of a block starts at 0, gaps filled by extending
    # the next piece down, last piece of a block extends to SEGBLK.
    pieces = []
    for b in range(nblk):
        plist = [p for p in raw if p[1] == b]
        assert plist, f"block {b} has no items"
        prev_end = 0
        for k, (t, _b, lo, hi) in enumerate(plist):
            lo = min(lo, prev_end)
            if k == 0:
                lo = 0
            if k == len(plist) - 1:
                hi = SEGBLK
            pieces.append([t, b, lo, hi - lo, 0, int(k == 0), int(k == len(plist) - 1)])
            prev_end = hi

    # A-blob columns
    a_off = 0
    for p in pieces:
        p[4] = a_off
        a_off += p[3]
    w_total = a_off

    a_blob = np.zeros((TILE, w_total), dtype=np.float16)
    for t, b, lo, w, aoff, _st, _fin in pieces:
        lo_i, hi_i = t * TILE, min((t + 1) * TILE, n_items)
        segs = seg_of_item[lo_i:hi_i]
        rel = segs - (b * SEGBLK + lo)
        rows = np.arange(hi_i - lo_i)
        m = (rel >= 0) & (rel < w)
        a_blob[rows[m], aoff + rel[m]] = RELU_SCALE
    return pieces, a_blob, nblk


# ---------------------------------------------------------------------------
def _build_kernel(n_groups, nblk, pieces, w_total, s_pad, apply_gb):
    """Build the Bass/Tile kernel. Structure must be identical across cores."""
    i_pad = n_groups * GROUP * TILE
    chunk_items = CHUNKG * GROUP * TILE
    n_chunks = (n_groups + CHUNKG - 1) // CHUNKG
    nc = bass.Bass()

    x_in = nc.dram_tensor("xt", [FDIM, i_pad], F16, kind="ExternalInput")
    xsum_in = nc.dram_tensor("xsum", [FDIM, s_pad], F16, kind="ExternalInput")
    wt_in = nc.dram_tensor("wt", [FDIM, BAG], F16, kind="ExternalInput")
    a_in = nc.dram_tensor("ablob", [TILE, w_total], F16, kind="ExternalInput")
    if apply_gb:
        gb_in = nc.dram_tensor("gammab", [128, 2, BAG], F16, kind="ExternalInput")
    out_t = nc.dram_tensor("out", [128, nblk * 4 * BAG], F16, kind="ExternalOutput")

    pieces_by_tile = {}
    for p in pieces:
        pieces_by_tile.setdefault(p[0], []).append(p)

    # relu engine pattern: greedy by accumulated engine time
    act_w, dve_w = RELU_ACT_SHARE, 1.0 - RELU_ACT_SHARE
    relu_eng = []
    la = ld = 0.0
    for g in range(n_groups):
        if la / act_w <= ld / dve_w:
            relu_eng.append("A")
            la += 1.0
        else:
            relu_eng.append("D")
            ld += 1.0

    with tile.TileContext(nc) as tc:
        with (
            tc.tile_pool(name="const", bufs=1) as const,
            tc.tile_pool(name="xp", bufs=3) as xp,
            tc.tile_pool(name="hp", bufs=6) as hp,
            tc.tile_pool(name="aggp", bufs=3) as aggp,
            tc.tile_pool(name="tpsp", bufs=6) as tpsp,
            tc.tile_pool(name="outp", bufs=3) as outp,
            tc.tile_pool(name="lnp", bufs=6) as lnp,
            tc.tile_pool(name="ps_h", bufs=2, space="PSUM") as ps_h,
            tc.tile_pool(name="ps_seg", bufs=2, space="PSUM") as ps_seg,
        ):
            wt_sb = const.tile([FDIM, BAG], F16, tag="wt")
            nc.sync.dma_start(wt_sb, wt_in[:])
            a_sb = const.tile([TILE, w_total], F16, tag="ablob")
            a_cut = min(2048, w_total)
            nc.sync.dma_start(a_sb[:, :a_cut], a_in[:, :a_cut])
            xsum_sb = const.tile([FDIM, s_pad], F16, tag="xsum")
            nc.sync.dma_start(xsum_sb[:, :1024], xsum_in[:, :1024])

            def emit_late_consts():
                nc.sync.dma_start(a_sb[:, a_cut:], a_in[:, a_cut:])
                if s_pad > 1024:
                    nc.sync.dma_start(xsum_sb[:, 1024:], xsum_in[:, 1024:])
            eps_sb = const.tile([128, 1], F32, tag="eps")
            nc.vector.memset(eps_sb, LN_EPS)
            if apply_gb:
                gb_sb = const.tile([128, 2, BAG], F16, tag="gb")
                nc.sync.dma_start(gb_sb, gb_in[:])

            seg_tiles = {}       # open block -> psum tile
            fin_f1 = []          # (block, seg_ps, due_group)
            fin_f2 = []          # (block, tps, due_group)
            fin_f3 = []          # (block, tps, mv, due_group)
            out_ring = {"tile": None, "b0": -1, "n": 0}
            x_cur = None
            h_tiles = {}         # group -> h_sb tile

            def emit_chunk(c):
                nonlocal x_cur
                lo = c * chunk_items
                hi = min(lo + chunk_items, i_pad)
                x_cur = xp.tile([FDIM, chunk_items], F16, tag="x", name=f"x{c}")
                q = XQ_PATTERN[c % len(XQ_PATTERN)]
                eng = {"s": nc.sync, "c": nc.scalar, "g": nc.gpsimd}[q]
                if c == 0:
                    gi = GROUP * TILE
                    for k in range(CHUNKG):
                        eng.dma_start(
                            x_cur[:, k * gi : (k + 1) * gi],
                            x_in[:, lo + k * gi : lo + (k + 1) * gi],
                        )
                else:
                    eng.dma_start(x_cur[:, : hi - lo], x_in[:, lo:hi])

            def emit_mm1_relu(g):
                h_ps = ps_h.tile([128, GROUP * 128], F32, tag="h", name=f"h{g}")
                base = (g % CHUNKG) * GROUP * 128
                for j in range(GROUP):
                    nc.tensor.matmul(
                        h_ps[:, j * 128 : (j + 1) * 128],
                        x_cur[:, base + j * 128 : base + (j + 1) * 128],
                        wt_sb,
                        start=True,
                        stop=True,
                    )
                h_sb = hp.tile([128, GROUP * 128], F16, tag="hs", name=f"hs{g}")
                if relu_eng[g] == "A":
                    nc.scalar.activation(
                        out=h_sb, in_=h_ps,
                        func=mybir.ActivationFunctionType.Relu,
                        bias=0.0, scale=1.0,
                    )
                else:
                    nc.vector.tensor_scalar_max(h_sb, h_ps, 0.0)
                h_tiles[g] = h_sb

            def emit_mm2(g):
                h_sb = h_tiles.pop(g)
                for j in range(GROUP):
                    t = g * GROUP + j
                    for (tt, b, lo, w, aoff, st, fin) in pieces_by_tile.get(t, []):
                        if b not in seg_tiles:
                            sp = ps_seg.tile([128, SEGBLK], F32, tag="seg",
                                             name=f"seg{b}")
                            seg_tiles[b] = sp
                            # linear term: 0.01*segsum(h) = wt @ (0.01*xsum)
                            nc.tensor.matmul(
                                sp, wt_sb,
                                xsum_sb[:, b * SEGBLK : (b + 1) * SEGBLK],
                                start=True, stop=False,
                                skip_group_check=True,
                            )
                        nc.tensor.matmul(
                            seg_tiles[b][:, lo : lo + w],
                            h_sb[:, j * 128 : (j + 1) * 128],
                            a_sb[:, aoff : aoff + w],
                            start=False,
                            stop=bool(fin),
                            skip_group_check=True,
                        )
                        if fin:
                            fin_f1.append([b, seg_tiles.pop(b), g])

            def emit_f1(b, seg_ps):
                agg = aggp.tile([128, SEGBLK], F16, tag="agg", name=f"agg{b}")
                nc.scalar.activation(
                    out=agg, in_=seg_ps,
                    func=mybir.ActivationFunctionType.Copy,
                    bias=0.0, scale=1.0,
                )
                tps = tpsp.tile([128, 4, 128], F16, tag="tps", name=f"tps{b}")
                nc.sync.dma_start_transpose(tps, agg)
                return tps

            def flush_out(force=False):
                r = out_ring
                if r["tile"] is None:
                    return
                if r["n"] == OUT_BATCH or force:
                    b0, n = r["b0"], r["n"]
                    nc.sync.dma_start(
                        out_t[:, b0 * 4 * BAG : (b0 + n) * 4 * BAG],
                        r["tile"][:, : n * 4, :],
                    )
                    r["tile"] = None
                    r["n"] = 0

            def emit_f2(b, tps):
                stats = lnp.tile([128, 4, 6], F32, tag="stats", name=f"st{b}")
                mv = lnp.tile([128, 4, 2], F32, tag="mv", name=f"mv{b}")
                for q in range(4):
                    nc.vector.bn_stats(stats[:, q, :], tps[:, q, :])
                    nc.vector.bn_aggr(mv[:, q, :], stats[:, q, :])
                return mv

            def emit_f3(b, tps, mv):
                rstd = lnp.tile([128, 4], F32, tag="rstd", name=f"rs{b}")
                nc.scalar.activation(
                    out=rstd, in_=mv[:, :, 1],
                    func=mybir.ActivationFunctionType.Sqrt,
                    bias=eps_sb[:, 0:1], scale=1.0,
                )
                nc.vector.reciprocal(rstd, rstd)
                r = out_ring
                if r["tile"] is None:
                    r["tile"] = outp.tile([128, OUT_BATCH * 4, BAG], F16,
                                          tag="out", name=f"ob{b}")
                    r["b0"] = b
                base = r["n"] * 4
                for q in range(4):
                    oq = r["tile"][:, base + q, :]
                    nc.gpsimd.tensor_scalar(
                        oq, tps[:, q, :], mv[:, q, 0:1], rstd[:, q : q + 1],
                        mybir.AluOpType.subtract, mybir.AluOpType.mult,
                    )
                    if apply_gb:
                        nc.vector.tensor_tensor(
                            oq, oq, gb_sb[:, 0, :], mybir.AluOpType.mult)
                        nc.vector.tensor_tensor(
                            oq, oq, gb_sb[:, 1, :], mybir.AluOpType.add)
                r["n"] += 1
                flush_out()

            def run_pending(g):
                while fin_f1 and fin_f1[0][2] + DEFER_F1 <= g:
                    b, sp, gd = fin_f1.pop(0)
                    tps = emit_f1(b, sp)
                    fin_f2.append([b, tps, gd])
                while fin_f2 and fin_f2[0][2] + DEFER_F2 <= g:
                    b, tps, gd = fin_f2.pop(0)
                    mv = emit_f2(b, tps)
                    fin_f3.append([b, tps, mv, gd])
                while fin_f3 and fin_f3[0][3] + DEFER_F3 <= g:
                    b, tps, mv, _gd = fin_f3.pop(0)
                    emit_f3(b, tps, mv)

            for g in range(n_groups):
                if g == 1:
                    emit_late_consts()
                if g % CHUNKG == 0:
                    emit_chunk(g // CHUNKG)
                emit_mm1_relu(g)
                if g >= DEFER_MM2:
                    emit_mm2(g - DEFER_MM2)
                run_pending(g)
            for g in range(n_groups - DEFER_MM2, n_groups):
                emit_mm2(g)
            run_pending(10 ** 9)
            flush_out(force=True)
    return _patch_bass(nc)


# ---------------------------------------------------------------------------
def kernel(feats, mask, W, b, gamma, beta, x_len):
    feats = np.asarray(feats, dtype=np.float32)
    mask = np.asarray(mask, dtype=np.float32)
    W = np.asarray(W, dtype=np.float32)
    b = np.asarray(b, dtype=np.float32)
    gamma = np.asarray(gamma, dtype=np.float32)
    beta = np.asarray(beta, dtype=np.float32)
    x_len = np.asarray(x_len, dtype=np.int32)

    n_seg = len(x_len)
    ends = np.cumsum(x_len, dtype=np.int64)

    # shard: equal contiguous segment ranges per core
    seg_bounds = [round(c * n_seg / NCORES) for c in range(NCORES + 1)]
    item_bounds = [0] + [int(ends[sb - 1]) if sb > 0 else 0 for sb in seg_bounds[1:]]

    core_lens = [x_len[seg_bounds[c] : seg_bounds[c + 1]] for c in range(NCORES)]
    core_items = [item_bounds[c + 1] - item_bounds[c] for c in range(NCORES)]

    group_items = TILE * GROUP
    i_pad = max(
        (max(core_items) + group_items - 1) // group_items * group_items,
        group_items,
    )

    structs = [_build_structure(cl, i_pad) for cl in core_lens]
    shapes_equal = all(
        structs[c][2] == structs[0][2]
        and len(structs[c][0]) == len(structs[0][0])
        and np.array_equal(np.array(structs[c][0]), np.array(structs[0][0]))
        for c in range(NCORES)
    )
    item_ranges = [(item_bounds[c], item_bounds[c + 1]) for c in range(NCORES)]
    replicated = not shapes_equal
    if replicated:
        # fallback: replicate the full problem on every core (slow, correct)
        n_items = int(ends[-1]) if n_seg else 0
        core_lens = [x_len] * NCORES
        item_ranges = [(0, n_items)] * NCORES
        i_pad = max(
            (n_items + group_items - 1) // group_items * group_items, group_items
        )
        st = _build_structure(x_len, i_pad)
        structs = [st] * NCORES

    pieces0, _, nblk = structs[0]
    n_groups = i_pad // group_items
    s_pad = nblk * SEGBLK
    w_total = pieces0[-1][4] + pieces0[-1][3]

    apply_gb = not (np.all(gamma == 1.0) and np.all(beta == 0.0))

    wt_aug = np.concatenate([W.T, b[None, :]], axis=0).astype(np.float16)  # [81,128]

    in_maps = []
    for c in range(NCORES):
        pieces, a_blob, _ = structs[c]
        i0, i1 = item_ranges[c]
        ni = i1 - i0
        # x transposed+augmented: [81, i_pad] f16
        xt = np.zeros((FDIM, i_pad), dtype=np.float16)
        xt[:FEAT, :ni] = feats[i0:i1].T
        xt[FEAT : FEAT + NMASK, :ni] = mask[i0:i1].T
        xt[FDIM - 1, :ni] = 1.0
        # per-segment input sums (scaled by LEAK), transposed: [81, s_pad] f16
        cl = core_lens[c].astype(np.int64)
        ns = len(cl)
        cs_f = np.concatenate(
            [np.zeros((1, FEAT + NMASK)),
             np.cumsum(np.concatenate([feats[i0:i1], mask[i0:i1]], axis=1,
                                      dtype=np.float64), axis=0)], axis=0)
        e = np.cumsum(cl)
        s = e - cl
        seg_sum = cs_f[e] - cs_f[s]          # [ns, 80]
        xsum = np.zeros((FDIM, s_pad), dtype=np.float16)
        xsum[:FEAT + NMASK, :ns] = (seg_sum.T * LEAK).astype(np.float16)
        xsum[FDIM - 1, :ns] = (cl * LEAK).astype(np.float16)
        im = {
            "xt": xt,
            "xsum": xsum,
            "wt": wt_aug,
            "ablob": a_blob,
        }
        if apply_gb:
            im["gammab"] = np.stack(
                [np.tile(gamma[None, :], (128, 1)), np.tile(beta[None, :], (128, 1))],
                axis=1,
            ).astype(np.float16)
        in_maps.append(im)

    nc = _build_kernel(n_groups, nblk, pieces0, w_total, s_pad, apply_gb)
    res = run_bass_kernel_spmd(nc, in_maps, core_ids=list(range(NCORES)))

    out = np.empty((n_seg, BAG), dtype=np.float32)
    for c in range(NCORES):
        lo, hi = seg_bounds[c], seg_bounds[c + 1]
        buf = res.results[c]["out"].reshape(128, nblk, 4, BAG)
        full = np.transpose(buf, (1, 2, 0, 3)).reshape(s_pad, BAG)
        out[lo:hi] = full[: hi - lo].astype(np.float32)
        if replicated:
            out[:] = full[:n_seg].astype(np.float32)
            break
    return out



# revision 24
# speedup vs baseline: 1.3997x; 1.3997x over previous
"""Trainium2 Bass kernel for nn_BagInput (segment_reduce), v3.

Pipeline per core (data-parallel over contiguous segment ranges):
  h    = x_aug @ (0.99*W_aug)     (PE; x host-pretransposed to [81, items] f16)
  r    = relu(h)                  (ACT activation Relu + DVE tensor_scalar_max;
                                   only these engines can read PSUM)
  seg  = segsum(r) + (0.01*W) @ xsum
       = A-matmul (0/1 fp8 blob) + wt01 @ xsumT (host per-seg sums, fp8)  (PE)
  out  = LayerNorm(seg):
           evict f16 (ACT) -> Pool square + partition-reduce (sum, sumsq)
           -> f16 stat rows -> tiny DMA-transpose -> per-partition scalars
           -> mu/var/rstd ([128,4] DVE + ACT sqrt) -> normalize (Pool)

LayerNorm is scale/shift-structure invariant per row, so segment SUMS replace
MEANS (the 1/len cancels).  lrelu decomposes as lrelu(h) = 0.01*h +
0.99*relu(h) = relu(0.99*h) + (0.01*W_aug) @ xsum_seg; the linear term uses
exact host-side per-segment input sums.

Scheduling: mm1 writes a 24-tile PSUM ring ([128, 3072] f32, 6 banks) in
8-tile windows with pipeline depth 3, so ACT and DVE run relu windows
back-to-back with no mm1 round-trip stall.  DMA transfers occupy their
issuing engine in the cost model, so x-chunks are split greedily between the
SP and Pool queues around their other work.
"""
import sys
import os

sys.path.insert(0, "/opt/trn_rl_repo")

import numpy as np
import ml_dtypes
import orjson

import concourse.bass as bass
import concourse.tile as tile
from concourse import mybir
from concourse.bass_utils import run_bass_kernel_spmd

FEAT = 64
NMASK = 16
FDIM = FEAT + NMASK + 1  # 81: feats + mask + ones column (bias)
BAG = 128
LEAK = 0.01
LN_EPS = 1e-5
NCORES = 8
TILE = 128            # items per tile (partition dim)
WIN = 8               # tiles per relu window
RING = 24             # tiles in the PSUM h ring (depth 3 windows)
CHUNKW = 2            # windows per input-DMA chunk
SEGBLK = 512          # segments per psum block
STAT_SCALE = 1.0 / 16.0   # f32->f16 stat row scaling (keeps sum(x^2) in range)
F16 = mybir.dt.float16
F32 = mybir.dt.float32
F8 = mybir.dt.float8e4

DEFER_MM2 = 2         # windows between relu emission and mm2 emission
DEFER_F1 = 0          # windows after block close before evict
DEFER_F1S = 5         # ... before square+reduces
DEFER_F1C = 6         # ... before converts + transposes
DEFER_F2 = 8          # ... before mu/var/rstd
DEFER_F3 = 10          # ... before normalize/out
OUT_BATCH = 4         # blocks per output DMA

# relu engine effective ns/window for greedy split (ACT : DVE)
RELU_COST_A = 1100.0
RELU_COST_D = 1200.0
RELU_PATTERN = "AD"   # e.g. "AD"; overrides the greedy split when set
EVICT_PATTERN = "A"   # per-block evict engine (A=ACT, D=DVE)
XLEAD = 4             # chunks of input-DMA lead
XBUFS = 5             # x chunk buffers
HP_BUFS = 4           # h_sb window buffers
AGG_BUFS = 3
SQ_BUFS = 2
RED_BUFS = 2
TST_BUFS = 2
TPS_BUFS = 4
LN_BUFS = 6
OUT_BUFS = 2
# per-block extra engine time, for the greedy DMA host split
DMA_CHUNK_COST = 3350.0


# ---------------------------------------------------------------------------
# BIR post-pass: this container's neuronxcc walrus accepts only ONE sync-wait
# per instruction; Tile attaches several.  Waiting on monotonic semaphores
# one-at-a-time in program order on the same engine is equivalent.
def _split_multi_waits(bir_bytes: bytes) -> bytes:
    mod = orjson.loads(bir_bytes)
    n = 0
    for fn in mod["functions"]:
        for bb in fn["blocks"]:
            out = []
            for ins in bb["instructions"]:
                si = ins.get("sync_info")
                waits = si.get("on_wait") if si else None
                if waits and len(waits) > 1:
                    for w in waits[:-1]:
                        n += 1
                        nop = {
                            "engine": ins["engine"],
                            "ins": [],
                            "name": f"WSPLIT-{n}",
                            "opcode": "NoOp",
                            "outs": [],
                            "sync_info": {"on_update": [], "on_wait": [w]},
                        }
                        if "debug" in ins:
                            nop["debug"] = ins["debug"]
                        out.append(nop)
                    si["on_wait"] = [waits[-1]]
                out.append(ins)
            bb["instructions"] = out
    return orjson.dumps(mod)


def _patch_bass(nc):
    orig = nc.to_json_bytes
    nc.to_json_bytes = lambda: _split_multi_waits(orig())
    return nc


# ---------------------------------------------------------------------------
# Host-side structure: per-core segment pieces for the segment-sum matmuls.
def _build_structure(x_len_core: np.ndarray, i_pad: int):
    """Pieces: [tile, block, psum_off, width, a_off, start, final] per
    128-item tile; windows cover every column of every 512-seg block.
    A entries are 1.0 (pure 0/1 indicator; scales live in wt99/wt01)."""
    x_len_core = x_len_core.astype(np.int64)
    n_items = int(x_len_core.sum())
    n_seg = len(x_len_core)
    nblk = (n_seg + SEGBLK - 1) // SEGBLK
    seg_of_item = np.repeat(np.arange(n_seg), x_len_core)
    t_pad = i_pad // TILE

    raw = []  # [tile, block, lo, hi) in block-local cols
    for t in range(t_pad):
        lo_i, hi_i = t * TILE, min((t + 1) * TILE, n_items)
        if lo_i >= n_items:
            break
        s0, s1 = int(seg_of_item[lo_i]), int(seg_of_item[hi_i - 1])
        for b in range(s0 // SEGBLK, s1 // SEGBLK + 1):
            sa, sb = max(s0, b * SEGBLK), min(s1, b * SEGBLK + SEGBLK - 1)
            raw.append([t, b, sa - b * SEGBLK, sb - b * SEGBLK + 1])

    # coverage: first piece of a block starts at 0, gaps filled by extending
    # the next piece down, last piece of a block extends to the block's
    # 128-rounded real width (SEGBLK except possibly the final block).
    pieces = []
    for b in range(nblk):
        blk_cols = min(SEGBLK, -(-(n_seg - b * SEGBLK) // 128) * 128)
        plist = [p for p in raw if p[1] == b]
        assert plist, f"block {b} has no items"
        prev_end = 0
        for k, (t, _b, lo, hi) in enumerate(plist):
            lo = min(lo, prev_end)
            if k == 0:
                lo = 0
            if k == len(plist) - 1:
                hi = blk_cols
            pieces.append([t, b, lo, hi - lo, 0, int(k == 0), int(k == len(plist) - 1)])
            prev_end = hi

    # A-blob columns
    a_off = 0
    for p in pieces:
        p[4] = a_off
        a_off += p[3]
    w_total = a_off

    a_blob = np.zeros((TILE, w_total), dtype=ml_dtypes.float8_e4m3fn)
    for t, b, lo, w, aoff, _st, _fin in pieces:
        lo_i, hi_i = t * TILE, min((t + 1) * TILE, n_items)
        segs = seg_of_item[lo_i:hi_i]
        rel = segs - (b * SEGBLK + lo)
        rows = np.arange(hi_i - lo_i)
        m = (rel >= 0) & (rel < w)
        a_blob[rows[m], aoff + rel[m]] = 1.0
    return pieces, a_blob, nblk


# ---------------------------------------------------------------------------
def _build_kernel(n_win, nblk, pieces, w_total, s_pad, apply_gb, n_seg_core):
    """Build the Bass/Tile kernel. Structure must be identical across cores."""
    qcount = [
        min(4, max(1, -(-(n_seg_core - b * SEGBLK) // 128)))
        for b in range(nblk)
    ]
    i_pad = n_win * WIN * TILE
    chunk_items = CHUNKW * WIN * TILE
    n_chunks = (n_win + CHUNKW - 1) // CHUNKW
    nc = bass.Bass()

    x_in = nc.dram_tensor("xt", [FDIM, i_pad], F16, kind="ExternalInput")
    xsum_in = nc.dram_tensor("xsum", [FDIM, s_pad], F8, kind="ExternalInput")
    wt99_in = nc.dram_tensor("wt99", [FDIM, BAG], F16, kind="ExternalInput")
    wt01_in = nc.dram_tensor("wt01", [FDIM, BAG], F16, kind="ExternalInput")
    a_in = nc.dram_tensor("ablob", [TILE, w_total], F8, kind="ExternalInput")
    if apply_gb:
        gb_in = nc.dram_tensor("gammab", [128, 2, BAG], F16, kind="ExternalInput")
    out_t = nc.dram_tensor("out", [128, nblk * 4 * BAG], F16, kind="ExternalOutput")

    pieces_by_tile = {}
    for p in pieces:
        pieces_by_tile.setdefault(p[0], []).append(p)

    # greedy relu engine split by accumulated time (ACT also pays block evict)
    if RELU_PATTERN:
        relu_eng = [RELU_PATTERN[w % len(RELU_PATTERN)] for w in range(n_win)]
    else:
        relu_eng = []
        la = ld = 0.0
        for w in range(n_win):
            if la <= ld:
                relu_eng.append("A")
                la += RELU_COST_A
                if (w * WIN * TILE) % (SEGBLK * 16) < WIN * TILE:
                    la += 716.0  # rough per-block evict share
            else:
                relu_eng.append("D")
                ld += RELU_COST_D

    with tile.TileContext(nc) as tc:
        with (
            tc.tile_pool(name="const", bufs=1) as const,
            tc.tile_pool(name="xp", bufs=XBUFS) as xp,
            tc.tile_pool(name="hp", bufs=HP_BUFS) as hp,
            tc.tile_pool(name="aggp", bufs=AGG_BUFS) as aggp,
            tc.tile_pool(name="sqp", bufs=SQ_BUFS) as sqp,
            tc.tile_pool(name="redp", bufs=RED_BUFS) as redp,
            tc.tile_pool(name="tstp", bufs=TST_BUFS) as tstp,
            tc.tile_pool(name="tpsp", bufs=TPS_BUFS) as tpsp,
            tc.tile_pool(name="lnp", bufs=LN_BUFS) as lnp,
            tc.tile_pool(name="outp", bufs=OUT_BUFS) as outp,
            tc.tile_pool(name="ps_h", bufs=3, space="PSUM") as ps_h,
            tc.tile_pool(name="ps_seg", bufs=2, space="PSUM") as ps_seg,
        ):
            # ---- engine DMA-load accumulators for greedy host choice ----
            load = {"s": 0.0, "g": 0.0}

            def dma_host(cost, prefer=None):
                q = prefer or ("s" if load["s"] <= load["g"] else "g")
                load[q] += cost
                return {"s": nc.sync, "g": nc.gpsimd}[q]

            wt99_sb = const.tile([FDIM, BAG], F16, tag="wt99")
            nc.sync.dma_start(wt99_sb, wt99_in[:])
            wt01_sb = const.tile([FDIM, BAG], F16, tag="wt01")
            nc.sync.dma_start(wt01_sb, wt01_in[:])
            load["s"] += 400.0
            a_sb = const.tile([TILE, w_total], F8, tag="ablob")
            a_cut = min(2048, w_total)
            nc.sync.dma_start(a_sb[:, :a_cut], a_in[:, :a_cut])
            load["s"] += a_cut * 0.39
            xsum_sb = const.tile([FDIM, s_pad], F8, tag="xsum")
            xs_cut = min(1024, s_pad)
            nc.sync.dma_start(xsum_sb[:, :xs_cut], xsum_in[:, :xs_cut])
            load["s"] += xs_cut * 0.39

            def emit_late_consts(step, nstep):
                for lo_, hi_, sb, dram in (
                    (a_cut, w_total, a_sb, a_in),
                    (xs_cut, s_pad, xsum_sb, xsum_in),
                ):
                    if hi_ <= lo_:
                        continue
                    p0 = lo_ + (hi_ - lo_) * step // nstep
                    p1 = lo_ + (hi_ - lo_) * (step + 1) // nstep
                    if p1 > p0:
                        eng = dma_host((p1 - p0) * 0.39)
                        eng.dma_start(sb[:, p0:p1], dram[:, p0:p1])

            eps_sb = const.tile([128, 1], F32, tag="eps")
            nc.vector.memset(eps_sb, LN_EPS)
            # persistent 16-row pads for the tiny stat transposes (rows 1-15
            # stay zero; only row 0 is rewritten per block)
            pA_sb = const.tile([16, SEGBLK], F16, tag="pA")
            nc.vector.memset(pA_sb, 0.0)
            pB_sb = const.tile([16, SEGBLK], F16, tag="pB")
            nc.vector.memset(pB_sb, 0.0)
            if apply_gb:
                gb_sb = const.tile([128, 2, BAG], F16, tag="gb")
                nc.sync.dma_start(gb_sb, gb_in[:])

            seg_tiles = {}       # open block -> psum tile
            fifo_f1 = []         # (block, seg_ps, due_win)
            fifo_f1s = []        # (block, agg, due)
            fifo_f1c = []        # (block, agg, stA, stB, due)
            fifo_f2 = []         # (block, tps, tpsA, tpsB, due)
            fifo_f3 = []         # (block, tps, mu, rstd, due)
            out_ring = {"tile": None, "b0": -1, "n": 0}
            x_tiles = {}         # chunk -> sbuf tile
            hps_tiles = {}       # window -> psum h tile
            h_tiles = {}         # window -> h_sb tile

            def emit_chunk(c, split=False):
                lo = c * chunk_items
                hi = min(lo + chunk_items, i_pad)
                xt_sb = xp.tile([FDIM, chunk_items], F16, tag="x", name=f"x{c}")
                x_tiles[c] = xt_sb
                if split:
                    wi = WIN * TILE
                    for k in range(CHUNKW):
                        eng = dma_host(wi * 0.78, prefer=("s", "g")[k % 2])
                        eng.dma_start(
                            xt_sb[:, k * wi : (k + 1) * wi],
                            x_in[:, lo + k * wi : lo + (k + 1) * wi],
                        )
                else:
                    eng = dma_host((hi - lo) * 0.78)
                    eng.dma_start(xt_sb[:, : hi - lo], x_in[:, lo:hi])

            def emit_mm1(w):
                c = (w * WIN * TILE) // chunk_items
                x_sb = x_tiles[c]
                base = (w * WIN * TILE) % chunk_items
                h_ps = ps_h.tile([128, WIN, TILE], F32, tag="h", name=f"h{w}")
                hps_tiles[w] = h_ps
                for j in range(WIN):
                    nc.tensor.matmul(
                        h_ps[:, j, :],
                        x_sb[:, base + j * TILE : base + (j + 1) * TILE],
                        wt99_sb,
                        start=True,
                        stop=True,
                    )

            def emit_relu(w):
                h_ps = hps_tiles.pop(w)
                h_sb = hp.tile([128, WIN, TILE], F16, tag="hs", name=f"hs{w}")
                if relu_eng[w] == "A":
                    nc.scalar.activation(
                        out=h_sb, in_=h_ps,
                        func=mybir.ActivationFunctionType.Relu,
                        bias=0.0, scale=1.0,
                    )
                else:
                    nc.vector.tensor_scalar_max(h_sb, h_ps, 0.0)
                h_tiles[w] = h_sb

            def emit_mm2(w):
                h_sb = h_tiles.pop(w)
                for j in range(WIN):
                    t = w * WIN + j
                    for (tt, b, lo, w_, aoff, st, fin) in pieces_by_tile.get(t, []):
                        if b not in seg_tiles:
                            sp = ps_seg.tile([128, SEGBLK], F32, tag="seg",
                                             name=f"seg{b}")
                            seg_tiles[b] = sp
                            # linear term: 0.01*segsum(h) = wt01 @ xsum
                            nc.tensor.matmul(
                                sp, wt01_sb,
                                xsum_sb[:, b * SEGBLK : (b + 1) * SEGBLK],
                                start=True, stop=False,
                                skip_group_check=True,
                            )
                        nc.tensor.matmul(
                            seg_tiles[b][:, lo : lo + w_],
                            h_sb[:, j, :],
                            a_sb[:, aoff : aoff + w_],
                            start=False,
                            stop=bool(fin),
                            skip_group_check=True,
                        )
                        if fin:
                            fifo_f1.append([b, seg_tiles.pop(b), w])

            def emit_f1(b, seg_ps):
                """Evict seg block psum -> f16 SBUF (ACT or DVE)."""
                cols = qcount[b] * 128
                agg = aggp.tile([128, SEGBLK], F16, tag="agg", name=f"agg{b}")
                if EVICT_PATTERN[b % len(EVICT_PATTERN)] == "A":
                    nc.scalar.activation(
                        out=agg[:, :cols], in_=seg_ps[:, :cols],
                        func=mybir.ActivationFunctionType.Copy,
                        bias=0.0, scale=1.0,
                    )
                else:
                    nc.vector.tensor_scalar(
                        agg[:, :cols], seg_ps[:, :cols], 0.0, None,
                        mybir.AluOpType.add)
                return agg

            def emit_f1s(b, agg):
                """Square + partition reductions (Pool)."""
                cols = qcount[b] * 128
                load["g"] += 2.5 * cols * 0.833
                sq = sqp.tile([128, SEGBLK], F16, tag="sq", name=f"sq{b}")
                nc.gpsimd.tensor_tensor(
                    sq[:, :cols], agg[:, :cols], agg[:, :cols],
                    mybir.AluOpType.mult)
                stA = redp.tile([1, SEGBLK], F32, tag="stA", name=f"stA{b}")
                nc.gpsimd.tensor_reduce(
                    stA[0:1, :cols], agg[:, :cols], mybir.AxisListType.C,
                    mybir.AluOpType.add)
                stB = redp.tile([1, SEGBLK], F32, tag="stB", name=f"stB{b}")
                nc.gpsimd.tensor_reduce(
                    stB[0:1, :cols], sq[:, :cols], mybir.AxisListType.C,
                    mybir.AluOpType.add)
                return stA, stB

            def emit_f1c(b, agg, stA, stB):
                """Convert stat rows to f16 (Pool), transpose stats + data (SP)."""
                qc = qcount[b]
                cols = qc * 128
                load["g"] += 2 * cols * 0.833
                nc.gpsimd.tensor_scalar(
                    pA_sb[0:1, :cols], stA[0:1, :cols], STAT_SCALE, None,
                    mybir.AluOpType.mult)
                nc.gpsimd.tensor_scalar(
                    pB_sb[0:1, :cols], stB[0:1, :cols], STAT_SCALE, None,
                    mybir.AluOpType.mult)
                tpsA = tstp.tile([128, 4, 16], F16, tag="tpsA", name=f"tpsA{b}")
                nc.sync.dma_start_transpose(tpsA[:, :qc, :], pA_sb[:, :cols])
                tpsB = tstp.tile([128, 4, 16], F16, tag="tpsB", name=f"tpsB{b}")
                nc.sync.dma_start_transpose(tpsB[:, :qc, :], pB_sb[:, :cols])
                tps = tpsp.tile([128, 4, TILE], F16, tag="tps", name=f"tps{b}")
                nc.sync.dma_start_transpose(tps[:, :qc, :], agg[:, :cols])
                load["s"] += 140.0 * qc
                return tps, tpsA, tpsB

            def emit_f2(b, tpsA, tpsB):
                """mu/var/rstd finish on [128,qc] tiles."""
                qc = qcount[b]
                mu = lnp.tile([128, 4], F32, tag="mu", name=f"mu{b}")
                nc.vector.tensor_scalar(
                    mu[:, :qc], tpsA[:, :qc, 0], 16.0 / BAG, None,
                    mybir.AluOpType.mult)
                musq = lnp.tile([128, 4], F32, tag="musq", name=f"musq{b}")
                nc.vector.tensor_tensor(
                    musq[:, :qc], mu[:, :qc], mu[:, :qc], mybir.AluOpType.mult)
                var = lnp.tile([128, 4], F32, tag="var", name=f"var{b}")
                nc.vector.tensor_scalar(
                    var[:, :qc], tpsB[:, :qc, 0], 16.0 / BAG, None,
                    mybir.AluOpType.mult)
                nc.vector.tensor_tensor(
                    var[:, :qc], var[:, :qc], musq[:, :qc],
                    mybir.AluOpType.subtract)
                rstd = lnp.tile([128, 4], F32, tag="rstd", name=f"rstd{b}")
                nc.scalar.activation(
                    out=rstd[:, :qc], in_=var[:, :qc],
                    func=mybir.ActivationFunctionType.Sqrt,
                    bias=eps_sb[:, 0:1], scale=1.0,
                )
                nc.vector.reciprocal(rstd[:, :qc], rstd[:, :qc])
                return mu, rstd

            def flush_out(force=False):
                r = out_ring
                if r["tile"] is None:
                    return
                if r["n"] == OUT_BATCH or force:
                    b0, n = r["b0"], r["n"]
                    eng = dma_host(n * 4 * BAG * 0.78)
                    eng.dma_start(
                        out_t[:, b0 * 4 * BAG : (b0 + n) * 4 * BAG],
                        r["tile"][:, : n * 4, :],
                    )
                    r["tile"] = None
                    r["n"] = 0

            def emit_f3(b, tps, mu, rstd):
                qc = qcount[b]
                load["g"] += 107.0 * qc
                if qc < 4:
                    # final short block: dedicated tile + immediate flush
                    flush_out(force=True)
                    ot = outp.tile([128, OUT_BATCH * 4, BAG], F16,
                                   tag="out", name=f"ob{b}")
                    for q in range(qc):
                        oq = ot[:, q, :]
                        nc.gpsimd.tensor_scalar(
                            oq, tps[:, q, :], mu[:, q : q + 1],
                            rstd[:, q : q + 1],
                            mybir.AluOpType.subtract, mybir.AluOpType.mult,
                        )
                        if apply_gb:
                            nc.vector.tensor_tensor(
                                oq, oq, gb_sb[:, 0, :], mybir.AluOpType.mult)
                            nc.vector.tensor_tensor(
                                oq, oq, gb_sb[:, 1, :], mybir.AluOpType.add)
                    eng = dma_host(qc * BAG * 0.78)
                    eng.dma_start(
                        out_t[:, b * 4 * BAG : b * 4 * BAG + qc * BAG],
                        ot[:, :qc, :],
                    )
                    return
                r = out_ring
                if r["tile"] is None:
                    r["tile"] = outp.tile([128, OUT_BATCH * 4, BAG], F16,
                                          tag="out", name=f"ob{b}")
                    r["b0"] = b
                base = r["n"] * 4
                for q in range(4):
                    oq = r["tile"][:, base + q, :]
                    nc.gpsimd.tensor_scalar(
                        oq, tps[:, q, :], mu[:, q : q + 1], rstd[:, q : q + 1],
                        mybir.AluOpType.subtract, mybir.AluOpType.mult,
                    )
                    if apply_gb:
                        nc.vector.tensor_tensor(
                            oq, oq, gb_sb[:, 0, :], mybir.AluOpType.mult)
                        nc.vector.tensor_tensor(
                            oq, oq, gb_sb[:, 1, :], mybir.AluOpType.add)
                r["n"] += 1
                flush_out()

            def run_pending(w):
                while fifo_f1 and fifo_f1[0][2] + DEFER_F1 <= w:
                    b, sp, wd = fifo_f1.pop(0)
                    agg = emit_f1(b, sp)
                    fifo_f1s.append([b, agg, wd])
                while fifo_f1s and fifo_f1s[0][2] + DEFER_F1S <= w:
                    b, agg, wd = fifo_f1s.pop(0)
                    stA, stB = emit_f1s(b, agg)
                    fifo_f1c.append([b, agg, stA, stB, wd])
                while fifo_f1c and fifo_f1c[0][4] + DEFER_F1C <= w:
                    b, agg, stA, stB, wd = fifo_f1c.pop(0)
                    tps, tpsA, tpsB = emit_f1c(b, agg, stA, stB)
                    fifo_f2.append([b, tps, tpsA, tpsB, wd])
                while fifo_f2 and fifo_f2[0][4] + DEFER_F2 <= w:
                    b, tps, tpsA, tpsB, wd = fifo_f2.pop(0)
                    mu, rstd = emit_f2(b, tpsA, tpsB)
                    fifo_f3.append([b, tps, mu, rstd, wd])
                while fifo_f3 and fifo_f3[0][4] + DEFER_F3 <= w:
                    b, tps, mu, rstd, _wd = fifo_f3.pop(0)
                    emit_f3(b, tps, mu, rstd)

            # ---- main loop ----
            emit_chunk(0, split=True)
            for c0 in range(1, min(XLEAD, n_chunks)):
                emit_chunk(c0, split=(c0 == 1))
            for w in range(n_win):
                if w in (1, 3, 5, 7):
                    emit_late_consts((w - 1) // 2, 4)
                if w % CHUNKW == 0:
                    c = w // CHUNKW + XLEAD
                    if c < n_chunks:
                        emit_chunk(c)
                emit_mm1(w)
                emit_relu(w)
                if w >= DEFER_MM2:
                    emit_mm2(w - DEFER_MM2)
                run_pending(w)
            for w in range(n_win - DEFER_MM2, n_win):
                emit_mm2(w)
            run_pending(10 ** 9)
            flush_out(force=True)
    return _patch_bass(nc)


# ---------------------------------------------------------------------------
def kernel(feats, mask, W, b, gamma, beta, x_len):
    feats = np.asarray(feats, dtype=np.float32)
    mask = np.asarray(mask, dtype=np.float32)
    W = np.asarray(W, dtype=np.float32)
    b = np.asarray(b, dtype=np.float32)
    gamma = np.asarray(gamma, dtype=np.float32)
    beta = np.asarray(beta, dtype=np.float32)
    x_len = np.asarray(x_len, dtype=np.int32)

    n_seg = len(x_len)
    ends = np.cumsum(x_len, dtype=np.int64)

    # shard: equal contiguous segment ranges per core
    seg_bounds = [round(c * n_seg / NCORES) for c in range(NCORES + 1)]
    item_bounds = [0] + [int(ends[sb - 1]) if sb > 0 else 0 for sb in seg_bounds[1:]]

    core_lens = [x_len[seg_bounds[c] : seg_bounds[c + 1]] for c in range(NCORES)]
    core_items = [item_bounds[c + 1] - item_bounds[c] for c in range(NCORES)]

    win_items = TILE * WIN
    i_pad = max(
        (max(core_items) + win_items - 1) // win_items * win_items,
        win_items * CHUNKW,
    )

    structs = [_build_structure(cl, i_pad) for cl in core_lens]
    shapes_equal = all(
        structs[c][2] == structs[0][2]
        and len(structs[c][0]) == len(structs[0][0])
        and np.array_equal(np.array(structs[c][0]), np.array(structs[0][0]))
        for c in range(NCORES)
    )
    item_ranges = [(item_bounds[c], item_bounds[c + 1]) for c in range(NCORES)]
    replicated = not shapes_equal
    if replicated:
        # fallback: replicate the full problem on every core (slow, correct)
        n_items = int(ends[-1]) if n_seg else 0
        core_lens = [x_len] * NCORES
        item_ranges = [(0, n_items)] * NCORES
        i_pad = max(
            (n_items + win_items - 1) // win_items * win_items,
            win_items * CHUNKW,
        )
        st = _build_structure(x_len, i_pad)
        structs = [st] * NCORES

    pieces0, _, nblk = structs[0]
    n_win = i_pad // win_items
    s_pad = nblk * SEGBLK
    w_total = pieces0[-1][4] + pieces0[-1][3]

    apply_gb = not (np.all(gamma == 1.0) and np.all(beta == 0.0))

    w_aug = np.concatenate([W.T, b[None, :]], axis=0)  # [81,128] f64-ish f32
    wt99 = (w_aug * (1.0 - LEAK)).astype(np.float16)
    wt01 = (w_aug * LEAK).astype(np.float16)

    in_maps = []
    for c in range(NCORES):
        pieces, a_blob, _ = structs[c]
        i0, i1 = item_ranges[c]
        ni = i1 - i0
        # x transposed+augmented: [81, i_pad] f16
        xt = np.zeros((FDIM, i_pad), dtype=np.float16)
        xt[:FEAT, :ni] = feats[i0:i1].T
        xt[FEAT : FEAT + NMASK, :ni] = mask[i0:i1].T
        xt[FDIM - 1, :ni] = 1.0
        # per-segment raw input sums, transposed: [81, s_pad] fp8
        cl = core_lens[c].astype(np.int64)
        ns = len(cl)
        cs_f = np.concatenate(
            [np.zeros((1, FEAT + NMASK)),
             np.cumsum(np.concatenate([feats[i0:i1], mask[i0:i1]], axis=1,
                                      dtype=np.float64), axis=0)], axis=0)
        e = np.cumsum(cl)
        s = e - cl
        seg_sum = cs_f[e] - cs_f[s]          # [ns, 80]
        xsum = np.zeros((FDIM, s_pad), dtype=ml_dtypes.float8_e4m3fn)
        xsum[:FEAT + NMASK, :ns] = seg_sum.T.astype(ml_dtypes.float8_e4m3fn)
        xsum[FDIM - 1, :ns] = cl.astype(ml_dtypes.float8_e4m3fn)
        im = {
            "xt": xt,
            "xsum": xsum,
            "wt99": wt99,
            "wt01": wt01,
            "ablob": a_blob,
        }
        if apply_gb:
            im["gammab"] = np.stack(
                [np.tile(gamma[None, :], (128, 1)), np.tile(beta[None, :], (128, 1))],
                axis=1,
            ).astype(np.float16)
        in_maps.append(im)

    n_seg_core0 = len(core_lens[0])
    nc = _build_kernel(n_win, nblk, pieces0, w_total, s_pad, apply_gb,
                       n_seg_core0)
    res = run_bass_kernel_spmd(nc, in_maps, core_ids=list(range(NCORES)))

    out = np.empty((n_seg, BAG), dtype=np.float32)
    for c in range(NCORES):
        lo, hi = seg_bounds[c], seg_bounds[c + 1]
        buf = res.results[c]["out"].reshape(128, nblk, 4, BAG)
        full = np.transpose(buf, (1, 2, 0, 3)).reshape(s_pad, BAG)
        out[lo:hi] = full[: hi - lo].astype(np.float32)
        if replicated:
            out[:] = full[:n_seg].astype(np.float32)
            break
    return out
